# revision 9
# baseline (speedup 1.0000x reference)
"""ConvAttention Trainium2 kernel (Bass/Tile), data-parallel over batch on 8
NeuronCores (1 batch per core, weights broadcast).

Reference computation (per batch b):
  q = conv3d(input, wq, 1x3x3, pad (0,1,1)) + bq, scaled by 0.5
  k = conv3d(memory, wk, 1x3x3, pad (0,1,1)) + bk
  v = conv3d(memory, wv, 3x3x3, pad (0,1,1)) + bv        (depth valid: L-2)
  heads split depth: q,k -> (2, 128, 9*32*32), v -> (2, 128, 8*32*32)
  logit[h] = q[h] @ k[h].T -> softmax over last axis -> @ v[h]
  out (128, 16, 32, 32)

Kernel design per core (v2 — minimal-matmul packing, fp16 data path):
  - All conv matmul time on the PE is output-row streaming (213ns per
    [128,512] fp32-accum matmul regardless of K), so the only lever is the
    NUMBER of matmuls: ceil(total_K / 128) per 512-position PSUM tile.
    Host stages shifted copies of each zero-padded [64, 34x34] depth slice so
    every matmul carries K=128 (two 64-channel taps):
      T1 = [P ; P<<1col]         -> q/k taps (dy,0)+(dy,1), v taps (dl,dy,0)+(dl,dy,1)
      T2 = [P<<2col ; P<<2col,1row] -> q/k pair (0,2)+(1,2); singles (2,2)
      T3 = [P_l<<2col ; P_(l+1)<<2col] -> v cross-depth pair (0,2,2)+(1,2,2)
    q/k: 5 matmuls per 16-row tile (vs 6 naive); v: 14 (vs 18). Total conv
    matmuls 808 + 32 attn@v + 144 logit = ~187us PE busy floor at 2.4GHz.
  - Whole data path in fp16 (inputs quantized on host; PSUM accum fp32):
    halves HBM traffic, keeps full PE rate; rel-err stays ~3e-3 << 2e-2.
  - One input DMA per tensor per slice (xa: T1|T2, ma: T1|T2|T3 staged
    contiguously in HBM) — no dependent on-chip shift copies, short HWDGE
    issue chain at startup.
  - PSUM evictions alternate DVE / Activation (Identity+bias AP) so neither
    engine gates PSUM recycling; attn output evictions rotate DVE/Act.
  - q,k conv outputs (bias fused) -> fp16 -> ONE blocked XBAR transpose per
    [128,1024] tile (out[p,j,c] = in[c,j*128+p]); the XBAR queue (nc.scalar)
    carries ONLY transposes.
  - logits accumulate per head in a persistent PSUM bank; each slice's logit
    matmuls are deferred one iteration so transposes hide behind conv work.
  - head 0 epilogue: softmax at l==9, its 16 attn@v chunks interleaved 4 per
    iteration into l=10..13 so PSUM evictions hide behind conv matmuls.
  - head 1: at l==17 the lv=15 v-conv is split around the final logit flush
    (tile0 -> flush -> tile1) so the l=17 transposes and the head-1 softmax
    both hide behind conv matmuls; attn@v chunks follow immediately.
  - outputs staged in [128,2048] fp16 tiles, ONE DMA per 4 chunks (8 total)
    to keep the tail short; host upcasts.

Timing note: per-iteration HW time is measured in test.py with a hardware
For_i loop (reps=257 vs 1) to cancel the axon dispatch overhead.
"""
import numpy as np

import concourse.bacc as bacc
import concourse.mybir as mybir
import concourse.tile as tile
from concourse import bass_utils

F32 = mybir.dt.float32
F16 = mybir.dt.float16

B, CIN, COUT, L, H, W = 8, 64, 128, 18, 32, 32
NH = 2              # heads
DQ = L // NH        # 9 depth slices per head for q/k
LV = L - 2          # 16 v depth slices
DV = LV // NH       # 8 per head
HP, WP = H + 2, W + 2          # padded spatial
SLICE = HP * WP                # 1156
NPOS = H * W                   # 1024 positions per depth slice
DEPTH_SCALE = 0.5

_CACHE = {}


def build_module(reps=1, **_legacy):
    """reps>1 wraps the whole computation in a hardware loop — used only for
    timing (amortizes the per-dispatch overhead of the execution path)."""
    nc = bacc.Bacc("TRN2", target_bir_lowering=False, debug=False)
    ACT = mybir.ActivationFunctionType

    xa = nc.dram_tensor("xa", [128, L, 2, SLICE], F16, kind="ExternalInput").ap()
    ma = nc.dram_tensor("ma", [128, L, 3, SLICE], F16, kind="ExternalInput").ap()
    # stationary packs: [K=128 (2 taps x 64ch), pass, M=128]
    wqk = nc.dram_tensor("wqk", [128, 10, 128], F16, kind="ExternalInput").ap()
    wv = nc.dram_tensor("wv", [128, 14, 128], F16, kind="ExternalInput").ap()
    bq = nc.dram_tensor("bq", [128, 1], F32, kind="ExternalInput").ap()
    bk = nc.dram_tensor("bk", [128, 1], F32, kind="ExternalInput").ap()
    bv = nc.dram_tensor("bv", [128, 1], F32, kind="ExternalInput").ap()
    out = nc.dram_tensor("out", [128, LV * NPOS], F16, kind="ExternalOutput").ap()

    with tile.TileContext(nc) as tc:
        with tc.tile_pool(name="consts", bufs=1) as cpool, \
             tc.tile_pool(name="xin", bufs=4) as xin_pool, \
             tc.tile_pool(name="xmem", bufs=6) as xmem_pool, \
             tc.tile_pool(name="qkc", bufs=6) as qkc_pool, \
             tc.tile_pool(name="qkT", bufs=6) as qkT_pool, \
             tc.tile_pool(name="vall", bufs=1) as vall_pool, \
             tc.tile_pool(name="sm", bufs=2) as sm_pool, \
             tc.tile_pool(name="ost", bufs=3) as ost_pool, \
             tc.tile_pool(name="pconv", bufs=6, space="PSUM") as pconv, \
             tc.tile_pool(name="plogit", bufs=1, space="PSUM") as plogit:

            wqk_t = cpool.tile([128, 10, 128], F16)
            bq_t = cpool.tile([128, 1], F32)
            bk_t = cpool.tile([128, 1], F32)
            bv_t = cpool.tile([128, 1], F32)
            wv_t = cpool.tile([128, 14, 128], F16)
            # all copy-mode DMAs ride the Pool SWDGE queue; nc.sync (SP) is
            # reserved for XBAR transposes (mode switches are global DMA
            # barriers, and a barrier-blocked transpose must not stall an
            # engine queue that has real work behind it)
            for t, d in [(wqk_t, wqk), (bq_t, bq), (bk_t, bk), (bv_t, bv),
                         (wv_t, wv)]:
                nc.gpsimd.dma_start(t[:], d)

            v_heads = [vall_pool.tile([128, DV * NPOS], F16, name=f"vh{h}")
                       for h in range(NH)]

            import contextlib
            rep_ctx = (tc.For_i(0, reps, 1) if reps > 1
                       else contextlib.nullcontext())
            with rep_ctx:
                logit_ps = [plogit.tile([128, 128], F32, tag="logit",
                                        name=f"logit{h}") for h in range(NH)]
                xa_w, ma_w = {}, {}

                def load_slice(l):
                    # Pool-engine SWDGE loads: keeps the HWDGE completion-sem
                    # ring free for the XBAR transposes (shared 8-lane ring
                    # otherwise stalls a transpose behind an input load)
                    xt = xin_pool.tile([128, 2, SLICE], F16, tag="xin",
                                       name="xin")
                    nc.gpsimd.dma_start(xt[:], xa[:, l])
                    mt = xmem_pool.tile([128, 3, SLICE], F16, tag="xmem",
                                        name="xmem")
                    nc.gpsimd.dma_start(mt[:], ma[:, l])
                    xa_w[l] = xt
                    ma_w[l] = mt

                def views(t, s, lo=0, hi=128):
                    """(lo:hi, section s) of a [128, n, SLICE] tile as p h w."""
                    return t[lo:hi, s].rearrange("p (h w) -> p h w", h=HP)

                def conv_q_tile(qp, xt, y0):
                    """5 matmuls: 3 T1 pairs, 1 T2 pair, 1 K=64 single (top)."""
                    t1 = views(xt, 0)
                    t2 = views(xt, 1)
                    t2t = views(xt, 1, 0, 64)
                    for dy in range(3):
                        nc.tensor.matmul(qp[:], wqk_t[:, dy],
                                         t1[:, y0 + dy:y0 + dy + 16, 0:32],
                                         start=(dy == 0), stop=False)
                    nc.tensor.matmul(qp[:], wqk_t[:, 3],
                                     t2[:, y0:y0 + 16, 0:32],
                                     start=False, stop=False)
                    nc.tensor.matmul(qp[:], wqk_t[0:64, 4],
                                     t2t[:, y0 + 2:y0 + 18, 0:32],
                                     start=False, stop=True)

                def conv_k_tile(kp, mt, y0):
                    """5 matmuls: 3 T1 pairs, 1 T2 pair, 1 K=64 single (bot:
                    T2 bottom holds P<<2,up1row, so rows y0+1 give tap (2,2))."""
                    t1 = views(mt, 0)
                    t2 = views(mt, 1)
                    t2b = views(mt, 1, 64, 128)
                    for dy in range(3):
                        nc.tensor.matmul(kp[:], wqk_t[:, 5 + dy],
                                         t1[:, y0 + dy:y0 + dy + 16, 0:32],
                                         start=(dy == 0), stop=False)
                    nc.tensor.matmul(kp[:], wqk_t[:, 8],
                                     t2[:, y0:y0 + 16, 0:32],
                                     start=False, stop=False)
                    nc.tensor.matmul(kp[:], wqk_t[64:128, 9],
                                     t2b[:, y0 + 1:y0 + 17, 0:32],
                                     start=False, stop=True)

                def conv_v_tile(vp, lv, y0):
                    """14 matmuls: 9 T1 pairs, 3 T2 pairs, 1 T3 cross-depth
                    pair ((0,2,2)+(1,2,2)), 1 K=64 single ((2,2,2))."""
                    for dl in range(3):
                        t1 = views(ma_w[lv + dl], 0)
                        for dy in range(3):
                            i = dl * 3 + dy
                            nc.tensor.matmul(vp[:], wv_t[:, i],
                                             t1[:, y0 + dy:y0 + dy + 16, 0:32],
                                             start=(i == 0), stop=False)
                    for dl in range(3):
                        t2 = views(ma_w[lv + dl], 1)
                        nc.tensor.matmul(vp[:], wv_t[:, 9 + dl],
                                         t2[:, y0:y0 + 16, 0:32],
                                         start=False, stop=False)
                    t3 = views(ma_w[lv], 2)
                    nc.tensor.matmul(vp[:], wv_t[:, 12],
                                     t3[:, y0 + 2:y0 + 18, 0:32],
                                     start=False, stop=False)
                    t2c = views(ma_w[lv + 2], 1, 0, 64)
                    nc.tensor.matmul(vp[:], wv_t[0:64, 13],
                                     t2c[:, y0 + 2:y0 + 18, 0:32],
                                     start=False, stop=True)

                def evict(dst, src, bias, use_act):
                    """PSUM -> SBUF fp16 with fused per-partition bias."""
                    if use_act:
                        nc.scalar.activation(dst, src, ACT.Identity,
                                             bias=bias)
                    else:
                        nc.vector.tensor_scalar_add(dst, src, bias)

                def conv_v_slice(lv, split_after_tile0=None):
                    """Both 16-row tiles of v output slice lv -> v_heads.
                    split_after_tile0: callback emitted between the tiles."""
                    vh, vd = lv // DV, lv % DV
                    for t in range(2):
                        vp = pconv.tile([128, 512], F32, tag="conv", name="vp")
                        conv_v_tile(vp, lv, t * 16)
                        evict(v_heads[vh][:, vd * NPOS + t * 512:
                                          vd * NPOS + (t + 1) * 512],
                              vp[:], bv_t[:], use_act=(t == 1))
                        if t == 0 and split_after_tile0 is not None:
                            split_after_tile0()

                def emit_logits(lslice, qT, kT):
                    hd = lslice // DQ
                    first = (lslice % DQ) == 0
                    last = (lslice % DQ) == DQ - 1
                    for j in range(8):
                        js = slice(j * 128, (j + 1) * 128)
                        nc.tensor.matmul(
                            logit_ps[hd][:], qT[:, js], kT[:, js],
                            start=(first and j == 0),
                            stop=(last and j == 7),
                            skip_group_check=True)

                attnT = {}

                def softmax_head(h):
                    negmax = sm_pool.tile([128, 1], F32, tag="negmax",
                                          name="negmax")
                    nc.vector.tensor_reduce(negmax[:], logit_ps[h][:],
                                            op=mybir.AluOpType.max,
                                            axis=mybir.AxisListType.X,
                                            negate=True)
                    attn_exp = sm_pool.tile([128, 128], F32, tag="attn_exp",
                                            name="attn_exp")
                    rowsum = sm_pool.tile([128, 1], F32, tag="rowsum",
                                          name="rowsum")
                    nc.scalar.activation(attn_exp[:], logit_ps[h][:],
                                         ACT.Exp, bias=negmax[:], scale=1.0,
                                         accum_out=rowsum[:])
                    recip = sm_pool.tile([128, 1], F32, tag="recip",
                                         name="recip")
                    nc.vector.reciprocal(recip[:], rowsum[:])
                    attn16 = sm_pool.tile([128, 128], F16, tag="attn16",
                                          name="attn16")
                    nc.vector.tensor_scalar_mul(attn16[:], attn_exp[:],
                                                recip[:])
                    aT = sm_pool.tile([128, 128], F16, tag="attnT",
                                      name="attnT")
                    nc.scalar.dma_start(aT[:], attn16[:], transpose=True)
                    attnT[h] = aT

                ost_cur = {}

                def attn_chunks(h, cs, stops=(3, 7, 11, 15)):
                    """attn@v for chunks cs of head h; multi-chunk output
                    groups staged in SBUF then stored with a single DMA.
                    `stops` sets group boundaries (smaller final groups keep
                    the kernel tail short)."""
                    group0 = {}
                    for c in cs:
                        if h not in group0 or group0[h] is None:
                            group0[h] = c
                            ost_cur[h] = ost_pool.tile([128, 2048], F16,
                                                       tag="ost", name="ost")
                        po = pconv.tile([128, 512], F32, tag="conv", name="po")
                        nc.tensor.matmul(po[:], attnT[h][:],
                                         v_heads[h][:, c * 512:(c + 1) * 512],
                                         start=True, stop=True)
                        g = c - group0[h]
                        dsl = ost_cur[h][:, g * 512:(g + 1) * 512]
                        if c % 2 == 0:
                            nc.vector.tensor_copy(dsl, po[:])
                        else:
                            nc.scalar.activation(dsl, po[:], ACT.Copy)
                        if c in stops:
                            off = h * DV * NPOS + group0[h] * 512
                            n = (g + 1) * 512
                            nc.sync.dma_start(out[:, off:off + n],
                                              ost_cur[h][:, 0:n])
                            group0[h] = None

                load_slice(0)
                load_slice(1)
                pending = None
                for l in range(L):
                    xt, mt = xa_w[l], ma_w[l]

                    qc = qkc_pool.tile([128, NPOS], F16, tag="qkc", name="qc")
                    kc = qkc_pool.tile([128, NPOS], F16, tag="qkc", name="kc")
                    for t in range(2):
                        sl = slice(t * 512, (t + 1) * 512)
                        qp = pconv.tile([128, 512], F32, tag="conv", name="qp")
                        conv_q_tile(qp, xt, t * 16)
                        evict(qc[:, sl], qp[:], bq_t[:], use_act=(t == 1))
                    for t in range(2):
                        sl = slice(t * 512, (t + 1) * 512)
                        kp = pconv.tile([128, 512], F32, tag="conv", name="kp")
                        conv_k_tile(kp, mt, t * 16)
                        evict(kc[:, sl], kp[:], bk_t[:], use_act=(t == 1))

                    qT = qkT_pool.tile([128, NPOS], F16, tag="qkT", name="qT")
                    kT = qkT_pool.tile([128, NPOS], F16, tag="qkT", name="kT")
                    for src_t, dst_t in ((qc, qT), (kc, kT)):
                        nc.scalar.dma_start_transpose(
                            dst_t[:].rearrange("p (j c) -> p j c", j=8),
                            src_t[:])

                    # flush the PREVIOUS slice's logits: its transposes have
                    # had a full slice of conv work to complete behind
                    if pending is not None:
                        emit_logits(*pending)
                    pending = (l, qT, kT)

                    if l == 9:
                        # head-0 logits flushed above (slice 8): emit softmax
                        # + attnT BEFORE this iter's loads so the attnT XBAR
                        # transpose doesn't barrier-wait on their transfers
                        softmax_head(0)

                    # issue the NEXT loads only after the transposes: an XBAR
                    # mode switch is a global DMA barrier, so a transpose
                    # waits for every regular DMA issued before it — loads
                    # issued here gate iter l+1's transposes (plenty of slack)
                    # instead of this iter's.
                    if l + 2 < L:
                        load_slice(l + 2)

                    if l < L - 1:
                        if l >= 2:
                            conv_v_slice(l - 2)
                        if 10 <= l <= 13:
                            c0 = 4 * (l - 10)
                            attn_chunks(0, range(c0, c0 + 4))
                    else:
                        # l == 17: split lv=15 v-conv around the final logit
                        # flush; softmax+attnT hide behind v tile1.
                        def _flush17():
                            emit_logits(*pending)
                        conv_v_slice(15, split_after_tile0=_flush17)
                        pending = None
                        softmax_head(1)
                        attn_chunks(1, range(16), stops=(3, 7, 11, 13, 15))
    nc.compile()
    return nc


def _shift_flat(flat, k):
    """flat [..., 1156] -> content shifted k positions earlier (zeros fill)."""
    out = np.zeros_like(flat)
    out[..., :SLICE - k] = flat[..., k:]
    return out


def prep_inputs(input, memory, wq, bq, wk, bk, wv, bv, **_legacy):
    """Host-side marshalling: fp16 shifted-copy image stages + weight packs."""
    input = np.asarray(input, dtype=np.float32)
    memory = np.asarray(memory, dtype=np.float32)
    wq = np.asarray(wq, dtype=np.float32) * DEPTH_SCALE
    bq = np.asarray(bq, dtype=np.float32) * DEPTH_SCALE
    wk = np.asarray(wk, dtype=np.float32)
    bk = np.asarray(bk, dtype=np.float32)
    wv = np.asarray(wv, dtype=np.float32)
    bv = np.asarray(bv, dtype=np.float32)

    def flat_padded(x):  # (B, CIN, L, SLICE) fp16
        p = np.zeros((B, CIN, L, HP, WP), np.float16)
        p[:, :, :, 1:H + 1, 1:W + 1] = x.astype(np.float16)
        return p.reshape(B, CIN, L, SLICE)

    def stage(flat, with_t3):
        # [B, 128, L, nsec, SLICE]
        nsec = 3 if with_t3 else 2
        st = np.zeros((B, 128, L, nsec, SLICE), np.float16)
        st[:, 0:64, :, 0] = flat
        st[:, 64:128, :, 0] = _shift_flat(flat, 1)
        t2top = _shift_flat(flat, 2)
        st[:, 0:64, :, 1] = t2top
        st[:, 64:128, :, 1] = _shift_flat(flat, HP + 2)
        if with_t3:
            st[:, 0:64, :, 2] = t2top
            st[:, 64:128, :L - 1, 2] = t2top[:, :, 1:]
        return st

    xa = stage(flat_padded(input), with_t3=False)
    ma = stage(flat_padded(memory), with_t3=True)

    def tap_qk(w, dy, dx):  # [64, 128] = (cin, cout)
        return w[:, :, 0, dy, dx].T

    wqk_p = np.zeros((128, 10, 128), np.float16)
    for dy in range(3):
        wqk_p[0:64, dy] = tap_qk(wq, dy, 0)
        wqk_p[64:128, dy] = tap_qk(wq, dy, 1)
        wqk_p[0:64, 5 + dy] = tap_qk(wk, dy, 0)
        wqk_p[64:128, 5 + dy] = tap_qk(wk, dy, 1)
    wqk_p[0:64, 3] = tap_qk(wq, 0, 2)
    wqk_p[64:128, 3] = tap_qk(wq, 1, 2)
    wqk_p[0:64, 4] = tap_qk(wq, 2, 2)
    wqk_p[0:64, 8] = tap_qk(wk, 0, 2)
    wqk_p[64:128, 8] = tap_qk(wk, 1, 2)
    wqk_p[64:128, 9] = tap_qk(wk, 2, 2)

    def tap_v(dl, dy, dx):
        return wv[:, :, dl, dy, dx].T

    wv_p = np.zeros((128, 14, 128), np.float16)
    for dl in range(3):
        for dy in range(3):
            wv_p[0:64, dl * 3 + dy] = tap_v(dl, dy, 0)
            wv_p[64:128, dl * 3 + dy] = tap_v(dl, dy, 1)
        wv_p[0:64, 9 + dl] = tap_v(dl, 0, 2)
        wv_p[64:128, 9 + dl] = tap_v(dl, 1, 2)
    wv_p[0:64, 12] = tap_v(0, 2, 2)
    wv_p[64:128, 12] = tap_v(1, 2, 2)
    wv_p[0:64, 13] = tap_v(2, 2, 2)

    shared = {
        "wqk": wqk_p, "wv": wv_p,
        "bq": bq.reshape(128, 1), "bk": bk.reshape(128, 1),
        "bv": bv.reshape(128, 1),
    }
    return [{"xa": np.ascontiguousarray(xa[b]),
             "ma": np.ascontiguousarray(ma[b]), **shared} for b in range(B)]


# legacy flags kept for test.py compatibility (ignored by build_module)
QK_F32R = True
SPLIT_LOGITS = False


def kernel(**inputs):
    if "nc" not in _CACHE:
        _CACHE["nc"] = build_module()
    nc = _CACHE["nc"]
    in_maps = prep_inputs(**inputs)
    res = bass_utils.run_bass_kernel_spmd(nc, in_maps, core_ids=list(range(B)))
    out = np.stack([res.results[b]["out"].reshape(COUT, LV, H, W)
                    for b in range(B)])
    return out.astype(np.float32)


# revision 19
# speedup vs baseline: 1.0604x; 1.0604x over previous
"""ConvAttention Trainium2 kernel (Bass/Tile), data-parallel over batch on 8
NeuronCores (1 batch per core, weights broadcast).

Reference computation (per batch b):
  q = conv3d(input, wq, 1x3x3, pad (0,1,1)) + bq, scaled by 0.5
  k = conv3d(memory, wk, 1x3x3, pad (0,1,1)) + bk
  v = conv3d(memory, wv, 3x3x3, pad (0,1,1)) + bv        (depth valid: L-2)
  heads split depth: q,k -> (2, 128, 9*32*32), v -> (2, 128, 8*32*32)
  logit[h] = q[h] @ k[h].T -> softmax over last axis -> @ v[h]
  out (128, 16, 32, 32)

Kernel design per core (v2 — minimal-matmul packing, fp16 data path):
  - All conv matmul time on the PE is output-row streaming (213ns per
    [128,512] fp32-accum matmul regardless of K), so the only lever is the
    NUMBER of matmuls: ceil(total_K / 128) per 512-position PSUM tile.
    Host stages shifted copies of each zero-padded [64, 34x34] depth slice so
    every matmul carries K=128 (two 64-channel taps):
      T1 = [P ; P<<1col]         -> q/k taps (dy,0)+(dy,1), v taps (dl,dy,0)+(dl,dy,1)
      T2 = [P<<2col ; P<<2col,1row] -> q/k pair (0,2)+(1,2); singles (2,2)
      T3 = [P_l<<2col ; P_(l+1)<<2col] -> v cross-depth pair (0,2,2)+(1,2,2)
    q/k: 5 matmuls per 16-row tile (vs 6 naive); v: 14 (vs 18). Total conv
    matmuls 808 + 32 attn@v + 144 logit = ~187us PE busy floor at 2.4GHz.
  - Whole data path in fp16 (inputs quantized on host; PSUM accum fp32):
    halves HBM traffic, keeps full PE rate; rel-err stays ~3e-3 << 2e-2.
  - One input DMA per tensor per slice (xa: T1|T2, ma: T1|T2|T3 staged
    contiguously in HBM) — no dependent on-chip shift copies, short HWDGE
    issue chain at startup.
  - PSUM evictions alternate DVE / Activation (Identity+bias AP) so neither
    engine gates PSUM recycling; attn output evictions rotate DVE/Act.
  - q,k conv outputs (bias fused) -> fp16 -> ONE blocked XBAR transpose per
    [128,1024] tile (out[p,j,c] = in[c,j*128+p]); the XBAR queue (nc.scalar)
    carries ONLY transposes.
  - logits accumulate per head in a persistent PSUM bank; each slice's logit
    matmuls are deferred one iteration so transposes hide behind conv work.
  - head 0 epilogue: softmax at l==9, its 16 attn@v chunks interleaved 4 per
    iteration into l=10..13 so PSUM evictions hide behind conv matmuls.
  - head 1: at l==17 the lv=15 v-conv is split around the final logit flush
    (tile0 -> flush -> tile1) so the l=17 transposes and the head-1 softmax
    both hide behind conv matmuls; attn@v chunks follow immediately.
  - outputs staged in [128,2048] fp16 tiles, ONE DMA per 4 chunks (8 total)
    to keep the tail short; host upcasts.

Timing note: per-iteration HW time is measured in test.py with a hardware
For_i loop (reps=257 vs 1) to cancel the axon dispatch overhead.
"""
import numpy as np

import concourse.bacc as bacc
import concourse.mybir as mybir
import concourse.tile as tile
from concourse import bass_utils

F32 = mybir.dt.float32
F16 = mybir.dt.float16

B, CIN, COUT, L, H, W = 8, 64, 128, 18, 32, 32
NH = 2              # heads
DQ = L // NH        # 9 depth slices per head for q/k
LV = L - 2          # 16 v depth slices
DV = LV // NH       # 8 per head
HP, WP = H + 2, W + 2          # padded spatial
SLICE = HP * WP                # 1156
NPOS = H * W                   # 1024 positions per depth slice
DEPTH_SCALE = 0.5

_CACHE = {}


def build_module(reps=1, **_legacy):
    """reps>1 wraps the whole computation in a hardware loop — used only for
    timing (amortizes the per-dispatch overhead of the execution path)."""
    nc = bacc.Bacc("TRN2", target_bir_lowering=False, debug=False)
    ACT = mybir.ActivationFunctionType

    xa = nc.dram_tensor("xa", [128, L, 2, SLICE], F16, kind="ExternalInput").ap()
    ma = nc.dram_tensor("ma", [128, L, 3, SLICE], F16, kind="ExternalInput").ap()
    # stationary packs: [K=128 (2 taps x 64ch), pass, M=128]
    wqk = nc.dram_tensor("wqk", [128, 10, 128], F16, kind="ExternalInput").ap()
    wv = nc.dram_tensor("wv", [128, 14, 128], F16, kind="ExternalInput").ap()
    bq = nc.dram_tensor("bq", [128, 1], F32, kind="ExternalInput").ap()
    bk = nc.dram_tensor("bk", [128, 1], F32, kind="ExternalInput").ap()
    bv = nc.dram_tensor("bv", [128, 1], F32, kind="ExternalInput").ap()
    out = nc.dram_tensor("out", [128, LV * NPOS], F16, kind="ExternalOutput").ap()

    with tile.TileContext(nc) as tc:
        with tc.tile_pool(name="consts", bufs=1) as cpool, \
             tc.tile_pool(name="xin", bufs=3) as xin_pool, \
             tc.tile_pool(name="xmem", bufs=5) as xmem_pool, \
             tc.tile_pool(name="qkc", bufs=6) as qkc_pool, \
             tc.tile_pool(name="qkT", bufs=6) as qkT_pool, \
             tc.tile_pool(name="vall", bufs=1) as vall_pool, \
             tc.tile_pool(name="sm", bufs=2) as sm_pool, \
             tc.tile_pool(name="ost", bufs=3) as ost_pool, \
             tc.tile_pool(name="pconv", bufs=6, space="PSUM") as pconv, \
             tc.tile_pool(name="plogit", bufs=1, space="PSUM") as plogit:

            wqk_t = cpool.tile([128, 10, 128], F16)
            bq_t = cpool.tile([128, 1], F32)
            bk_t = cpool.tile([128, 1], F32)
            bv_t = cpool.tile([128, 1], F32)
            wv_t = cpool.tile([128, 14, 128], F16)
            # queue layout: nc.sync (SP) carries ONLY XBAR transposes (a mode
            # switch is a global DMA barrier; a barrier-blocked transpose must
            # not stall a queue with real work behind it). Weight loads and
            # output stores ride the Act HWDGE queue (copies only); per-slice
            # input loads ride the Pool SWDGE queue (separate completion-sem
            # ring, so transposes never queue behind a big input transfer).
            for t, d in [(wqk_t, wqk), (bq_t, bq), (bk_t, bk), (bv_t, bv),
                         (wv_t, wv)]:
                nc.scalar.dma_start(t[:], d)

            v_heads = [vall_pool.tile([128, DV * NPOS], F16, name=f"vh{h}")
                       for h in range(NH)]

            import contextlib
            rep_ctx = (tc.For_i(0, reps, 1) if reps > 1
                       else contextlib.nullcontext())
            with rep_ctx:
                logit_ps = [plogit.tile([128, 128], F32, tag="logit",
                                        name=f"logit{h}") for h in range(NH)]
                xa_w, ma_w = {}, {}

                def load_slice(l):
                    # Pool-engine SWDGE loads: keeps the HWDGE completion-sem
                    # ring free for the XBAR transposes (shared 8-lane ring
                    # otherwise stalls a transpose behind an input load)
                    xt = xin_pool.tile([128, 2, SLICE], F16, tag="xin",
                                       name="xin")
                    nc.gpsimd.dma_start(xt[:], xa[:, l])
                    mt = xmem_pool.tile([128, 3, SLICE], F16, tag="xmem",
                                        name="xmem")
                    nc.gpsimd.dma_start(mt[:], ma[:, l])
                    xa_w[l] = xt
                    ma_w[l] = mt

                def views(t, s, lo=0, hi=128):
                    """(lo:hi, section s) of a [128, n, SLICE] tile as p h w."""
                    return t[lo:hi, s].rearrange("p (h w) -> p h w", h=HP)

                def conv_q_tile(qp, xt, y0):
                    """5 matmuls: 3 T1 pairs, 1 T2 pair, 1 K=64 single (top)."""
                    t1 = views(xt, 0)
                    t2 = views(xt, 1)
                    t2t = views(xt, 1, 0, 64)
                    for dy in range(3):
                        nc.tensor.matmul(qp[:], wqk_t[:, dy],
                                         t1[:, y0 + dy:y0 + dy + 16, 0:32],
                                         start=(dy == 0), stop=False)
                    nc.tensor.matmul(qp[:], wqk_t[:, 3],
                                     t2[:, y0:y0 + 16, 0:32],
                                     start=False, stop=False)
                    nc.tensor.matmul(qp[:], wqk_t[0:64, 4],
                                     t2t[:, y0 + 2:y0 + 18, 0:32],
                                     start=False, stop=True)

                def conv_k_tile(kp, mt, y0):
                    """5 matmuls: 3 T1 pairs, 1 T2 pair, 1 K=64 single (bot:
                    T2 bottom holds P<<2,up1row, so rows y0+1 give tap (2,2))."""
                    t1 = views(mt, 0)
                    t2 = views(mt, 1)
                    t2b = views(mt, 1, 64, 128)
                    for dy in range(3):
                        nc.tensor.matmul(kp[:], wqk_t[:, 5 + dy],
                                         t1[:, y0 + dy:y0 + dy + 16, 0:32],
                                         start=(dy == 0), stop=False)
                    nc.tensor.matmul(kp[:], wqk_t[:, 8],
                                     t2[:, y0:y0 + 16, 0:32],
                                     start=False, stop=False)
                    nc.tensor.matmul(kp[:], wqk_t[64:128, 9],
                                     t2b[:, y0 + 1:y0 + 17, 0:32],
                                     start=False, stop=True)

                def conv_v_tile(vp, lv, y0):
                    """14 matmuls: 9 T1 pairs, 3 T2 pairs, 1 T3 cross-depth
                    pair ((0,2,2)+(1,2,2)), 1 K=64 single ((2,2,2))."""
                    for dl in range(3):
                        t1 = views(ma_w[lv + dl], 0)
                        for dy in range(3):
                            i = dl * 3 + dy
                            nc.tensor.matmul(vp[:], wv_t[:, i],
                                             t1[:, y0 + dy:y0 + dy + 16, 0:32],
                                             start=(i == 0), stop=False)
                    for dl in range(3):
                        t2 = views(ma_w[lv + dl], 1)
                        nc.tensor.matmul(vp[:], wv_t[:, 9 + dl],
                                         t2[:, y0:y0 + 16, 0:32],
                                         start=False, stop=False)
                    t3 = views(ma_w[lv], 2)
                    nc.tensor.matmul(vp[:], wv_t[:, 12],
                                     t3[:, y0 + 2:y0 + 18, 0:32],
                                     start=False, stop=False)
                    t2c = views(ma_w[lv + 2], 1, 0, 64)
                    nc.tensor.matmul(vp[:], wv_t[0:64, 13],
                                     t2c[:, y0 + 2:y0 + 18, 0:32],
                                     start=False, stop=True)

                def evict(dst, src, bias, use_act):
                    """PSUM -> SBUF fp16 with fused per-partition bias."""
                    if use_act:
                        nc.scalar.activation(dst, src, ACT.Identity,
                                             bias=bias)
                    else:
                        nc.vector.tensor_scalar_add(dst, src, bias)

                def conv_v_slice(lv, split_after_tile0=None):
                    """Both 16-row tiles of v output slice lv -> v_heads.
                    split_after_tile0: callback emitted between the tiles."""
                    vh, vd = lv // DV, lv % DV
                    for t in range(2):
                        vp = pconv.tile([128, 512], F32, tag="conv", name="vp")
                        conv_v_tile(vp, lv, t * 16)
                        evict(v_heads[vh][:, vd * NPOS + t * 512:
                                          vd * NPOS + (t + 1) * 512],
                              vp[:], bv_t[:], use_act=(t == 1))
                        if t == 0 and split_after_tile0 is not None:
                            split_after_tile0()

                def emit_logits(lslice, qkT):
                    hd = lslice // DQ
                    first = (lslice % DQ) == 0
                    last = (lslice % DQ) == DQ - 1
                    for j in range(8):
                        qs = slice(j * 128, (j + 1) * 128)
                        ks = slice(NPOS + j * 128, NPOS + (j + 1) * 128)
                        nc.tensor.matmul(
                            logit_ps[hd][:], qkT[:, qs], qkT[:, ks],
                            start=(first and j == 0),
                            stop=(last and j == 7),
                            skip_group_check=True)

                attnT = {}

                def softmax_head(h):
                    negmax = sm_pool.tile([128, 1], F32, tag="negmax",
                                          name="negmax")
                    nc.vector.tensor_reduce(negmax[:], logit_ps[h][:],
                                            op=mybir.AluOpType.max,
                                            axis=mybir.AxisListType.X,
                                            negate=True)
                    attn_exp = sm_pool.tile([128, 128], F32, tag="attn_exp",
                                            name="attn_exp")
                    rowsum = sm_pool.tile([128, 1], F32, tag="rowsum",
                                          name="rowsum")
                    nc.scalar.activation(attn_exp[:], logit_ps[h][:],
                                         ACT.Exp, bias=negmax[:], scale=1.0,
                                         accum_out=rowsum[:])
                    recip = sm_pool.tile([128, 1], F32, tag="recip",
                                         name="recip")
                    nc.vector.reciprocal(recip[:], rowsum[:])
                    attn16 = sm_pool.tile([128, 128], F16, tag="attn16",
                                          name="attn16")
                    nc.vector.tensor_scalar_mul(attn16[:], attn_exp[:],
                                                recip[:])
                    aT = sm_pool.tile([128, 128], F16, tag="attnT",
                                      name="attnT")
                    nc.sync.dma_start(aT[:], attn16[:], transpose=True)
                    attnT[h] = aT

                ost_cur = {}

                def attn_chunks(h, cs, stops=(3, 7, 11, 15)):
                    """attn@v for chunks cs of head h; multi-chunk output
                    groups staged in SBUF then stored with a single DMA.
                    `stops` sets group boundaries (smaller final groups keep
                    the kernel tail short)."""
                    group0 = {}
                    for c in cs:
                        if h not in group0 or group0[h] is None:
                            group0[h] = c
                            ost_cur[h] = ost_pool.tile([128, 2048], F16,
                                                       tag="ost", name="ost")
                        po = pconv.tile([128, 512], F32, tag="conv", name="po")
                        nc.tensor.matmul(po[:], attnT[h][:],
                                         v_heads[h][:, c * 512:(c + 1) * 512],
                                         start=True, stop=True)
                        g = c - group0[h]
                        dsl = ost_cur[h][:, g * 512:(g + 1) * 512]
                        if c % 2 == 0:
                            nc.vector.tensor_copy(dsl, po[:])
                        else:
                            nc.scalar.activation(dsl, po[:], ACT.Copy)
                        if c in stops:
                            off = h * DV * NPOS + group0[h] * 512
                            n = (g + 1) * 512
                            nc.scalar.dma_start(out[:, off:off + n],
                                                ost_cur[h][:, 0:n])
                            group0[h] = None

                load_slice(0)
                load_slice(1)
                pending = None
                for l in range(L):
                    xt, mt = xa_w[l], ma_w[l]

                    # q and k conv outputs share ONE [128, 2048] staging tile
                    # so a single XBAR transpose (one global-barrier mode
                    # switch) covers both
                    qkc = qkc_pool.tile([128, 2 * NPOS], F16, tag="qkc",
                                        name="qkc")
                    for t in range(2):
                        sl = slice(t * 512, (t + 1) * 512)
                        qp = pconv.tile([128, 512], F32, tag="conv", name="qp")
                        conv_q_tile(qp, xt, t * 16)
                        evict(qkc[:, sl], qp[:], bq_t[:], use_act=(t == 1))
                    for t in range(2):
                        sl = slice(NPOS + t * 512, NPOS + (t + 1) * 512)
                        kp = pconv.tile([128, 512], F32, tag="conv", name="kp")
                        conv_k_tile(kp, mt, t * 16)
                        evict(qkc[:, sl], kp[:], bk_t[:], use_act=(t == 1))

                    qkT = qkT_pool.tile([128, 2 * NPOS], F16, tag="qkT",
                                        name="qkT")
                    nc.sync.dma_start_transpose(
                        qkT[:].rearrange("p (j c) -> p j c", j=16), qkc[:])

                    # flush the PREVIOUS slice's logits: its transposes have
                    # had a full slice of conv work to complete behind
                    if pending is not None:
                        emit_logits(*pending)
                    pending = (l, qkT)

                    if l == 9:
                        # head-0 logits flushed above (slice 8): emit softmax
                        # + attnT BEFORE this iter's loads so the attnT XBAR
                        # transpose doesn't barrier-wait on their transfers
                        softmax_head(0)

                    # issue the NEXT loads only after the transposes: an XBAR
                    # mode switch is a global DMA barrier, so a transpose
                    # waits for every regular DMA issued before it — loads
                    # issued here gate iter l+1's transposes (plenty of slack)
                    # instead of this iter's.
                    if l + 2 < L:
                        load_slice(l + 2)

                    if l < L - 1:
                        if l >= 2:
                            conv_v_slice(l - 2)
                        if 10 <= l <= 13:
                            c0 = 4 * (l - 10)
                            attn_chunks(0, range(c0, c0 + 4))
                    else:
                        # l == 17: split lv=15 v-conv around the final logit
                        # flush + head-1 softmax, so the attnT XBAR latency
                        # hides behind v tile1's matmuls.
                        def _flush17():
                            emit_logits(*pending)
                            softmax_head(1)
                        conv_v_slice(15, split_after_tile0=_flush17)
                        pending = None
                        attn_chunks(1, range(16), stops=(3, 7, 11, 13, 15))
    nc.compile()
    return nc


def _shift_flat(flat, k):
    """flat [..., 1156] -> content shifted k positions earlier (zeros fill)."""
    out = np.zeros_like(flat)
    out[..., :SLICE - k] = flat[..., k:]
    return out


def prep_inputs(input, memory, wq, bq, wk, bk, wv, bv, **_legacy):
    """Host-side marshalling: fp16 shifted-copy image stages + weight packs."""
    input = np.asarray(input, dtype=np.float32)
    memory = np.asarray(memory, dtype=np.float32)
    wq = np.asarray(wq, dtype=np.float32) * DEPTH_SCALE
    bq = np.asarray(bq, dtype=np.float32) * DEPTH_SCALE
    wk = np.asarray(wk, dtype=np.float32)
    bk = np.asarray(bk, dtype=np.float32)
    wv = np.asarray(wv, dtype=np.float32)
    bv = np.asarray(bv, dtype=np.float32)

    def flat_padded(x):  # (B, CIN, L, SLICE) fp16
        p = np.zeros((B, CIN, L, HP, WP), np.float16)
        p[:, :, :, 1:H + 1, 1:W + 1] = x.astype(np.float16)
        return p.reshape(B, CIN, L, SLICE)

    def stage(flat, with_t3):
        # [B, 128, L, nsec, SLICE]
        nsec = 3 if with_t3 else 2
        st = np.zeros((B, 128, L, nsec, SLICE), np.float16)
        st[:, 0:64, :, 0] = flat
        st[:, 64:128, :, 0] = _shift_flat(flat, 1)
        t2top = _shift_flat(flat, 2)
        st[:, 0:64, :, 1] = t2top
        st[:, 64:128, :, 1] = _shift_flat(flat, HP + 2)
        if with_t3:
            st[:, 0:64, :, 2] = t2top
            st[:, 64:128, :L - 1, 2] = t2top[:, :, 1:]
        return st

    xa = stage(flat_padded(input), with_t3=False)
    ma = stage(flat_padded(memory), with_t3=True)

    def tap_qk(w, dy, dx):  # [64, 128] = (cin, cout)
        return w[:, :, 0, dy, dx].T

    wqk_p = np.zeros((128, 10, 128), np.float16)
    for dy in range(3):
        wqk_p[0:64, dy] = tap_qk(wq, dy, 0)
        wqk_p[64:128, dy] = tap_qk(wq, dy, 1)
        wqk_p[0:64, 5 + dy] = tap_qk(wk, dy, 0)
        wqk_p[64:128, 5 + dy] = tap_qk(wk, dy, 1)
    wqk_p[0:64, 3] = tap_qk(wq, 0, 2)
    wqk_p[64:128, 3] = tap_qk(wq, 1, 2)
    wqk_p[0:64, 4] = tap_qk(wq, 2, 2)
    wqk_p[0:64, 8] = tap_qk(wk, 0, 2)
    wqk_p[64:128, 8] = tap_qk(wk, 1, 2)
    wqk_p[64:128, 9] = tap_qk(wk, 2, 2)

    def tap_v(dl, dy, dx):
        return wv[:, :, dl, dy, dx].T

    wv_p = np.zeros((128, 14, 128), np.float16)
    for dl in range(3):
        for dy in range(3):
            wv_p[0:64, dl * 3 + dy] = tap_v(dl, dy, 0)
            wv_p[64:128, dl * 3 + dy] = tap_v(dl, dy, 1)
        wv_p[0:64, 9 + dl] = tap_v(dl, 0, 2)
        wv_p[64:128, 9 + dl] = tap_v(dl, 1, 2)
    wv_p[0:64, 12] = tap_v(0, 2, 2)
    wv_p[64:128, 12] = tap_v(1, 2, 2)
    wv_p[0:64, 13] = tap_v(2, 2, 2)

    shared = {
        "wqk": wqk_p, "wv": wv_p,
        "bq": bq.reshape(128, 1), "bk": bk.reshape(128, 1),
        "bv": bv.reshape(128, 1),
    }
    return [{"xa": np.ascontiguousarray(xa[b]),
             "ma": np.ascontiguousarray(ma[b]), **shared} for b in range(B)]


# legacy flags kept for test.py compatibility (ignored by build_module)
QK_F32R = True
SPLIT_LOGITS = False


def kernel(**inputs):
    if "nc" not in _CACHE:
        _CACHE["nc"] = build_module()
    nc = _CACHE["nc"]
    in_maps = prep_inputs(**inputs)
    res = bass_utils.run_bass_kernel_spmd(nc, in_maps, core_ids=list(range(B)))
    out = np.stack([res.results[b]["out"].reshape(COUT, LV, H, W)
                    for b in range(B)])
    return out.astype(np.float32)


# revision 23
# speedup vs baseline: 1.0808x; 1.0192x over previous
"""ConvAttention Trainium2 kernel (Bass/Tile), data-parallel over batch on 8
NeuronCores (1 batch per core, weights broadcast).

Reference computation (per batch b):
  q = conv3d(input, wq, 1x3x3, pad (0,1,1)) + bq, scaled by 0.5
  k = conv3d(memory, wk, 1x3x3, pad (0,1,1)) + bk
  v = conv3d(memory, wv, 3x3x3, pad (0,1,1)) + bv        (depth valid: L-2)
  heads split depth: q,k -> (2, 128, 9*32*32), v -> (2, 128, 8*32*32)
  logit[h] = q[h] @ k[h].T -> softmax over last axis -> @ v[h]
  out (128, 16, 32, 32)

Kernel design per core (v2 — minimal-matmul packing, fp16 data path):
  - All conv matmul time on the PE is output-row streaming (213ns per
    [128,512] fp32-accum matmul regardless of K), so the only lever is the
    NUMBER of matmuls: ceil(total_K / 128) per 512-position PSUM tile.
    Host stages shifted copies of each zero-padded [64, 34x34] depth slice so
    every matmul carries K=128 (two 64-channel taps):
      T1 = [P ; P<<1col]         -> q/k taps (dy,0)+(dy,1), v taps (dl,dy,0)+(dl,dy,1)
      T2 = [P<<2col ; P<<2col,1row] -> q/k pair (0,2)+(1,2); singles (2,2)
      T3 = [P_l<<2col ; P_(l+1)<<2col] -> v cross-depth pair (0,2,2)+(1,2,2)
    q/k: 5 matmuls per 16-row tile (vs 6 naive); v: 14 (vs 18). Total conv
    matmuls 808 + 32 attn@v + 144 logit = ~187us PE busy floor at 2.4GHz.
  - Whole data path in fp16 (inputs quantized on host; PSUM accum fp32):
    halves HBM traffic, keeps full PE rate; rel-err stays ~3e-3 << 2e-2.
  - One input DMA per tensor per slice (xa: T1|T2, ma: T1|T2|T3 staged
    contiguously in HBM) — no dependent on-chip shift copies, short HWDGE
    issue chain at startup.
  - PSUM evictions alternate DVE / Activation (Identity+bias AP) so neither
    engine gates PSUM recycling; attn output evictions rotate DVE/Act.
  - q,k conv outputs (bias fused) -> fp16 -> ONE blocked XBAR transpose per
    [128,1024] tile (out[p,j,c] = in[c,j*128+p]); the XBAR queue (nc.scalar)
    carries ONLY transposes.
  - logits accumulate per head in a persistent PSUM bank; each slice's logit
    matmuls are deferred one iteration so transposes hide behind conv work.
  - head 0 epilogue: softmax at l==9, its 16 attn@v chunks interleaved 4 per
    iteration into l=10..13 so PSUM evictions hide behind conv matmuls.
  - head 1: at l==17 the lv=15 v-conv is split around the final logit flush
    (tile0 -> flush -> tile1) so the l=17 transposes and the head-1 softmax
    both hide behind conv matmuls; attn@v chunks follow immediately.
  - outputs staged in [128,2048] fp16 tiles, ONE DMA per 4 chunks (8 total)
    to keep the tail short; host upcasts.

Timing note: per-iteration HW time is measured in test.py with a hardware
For_i loop (reps=257 vs 1) to cancel the axon dispatch overhead.
"""
import numpy as np

import concourse.bacc as bacc
import concourse.mybir as mybir
import concourse.tile as tile
from concourse import bass_utils

F32 = mybir.dt.float32
F16 = mybir.dt.float16

B, CIN, COUT, L, H, W = 8, 64, 128, 18, 32, 32
NH = 2              # heads
DQ = L // NH        # 9 depth slices per head for q/k
LV = L - 2          # 16 v depth slices
DV = LV // NH       # 8 per head
HP, WP = H + 2, W + 2          # padded spatial
SLICE = HP * WP                # 1156
NPOS = H * W                   # 1024 positions per depth slice
DEPTH_SCALE = 0.5

_CACHE = {}


def build_module(reps=1, **_legacy):
    """reps>1 wraps the whole computation in a hardware loop — used only for
    timing (amortizes the per-dispatch overhead of the execution path)."""
    nc = bacc.Bacc("TRN2", target_bir_lowering=False, debug=False)
    ACT = mybir.ActivationFunctionType

    xa = nc.dram_tensor("xa", [128, L, 2, SLICE], F16, kind="ExternalInput").ap()
    ma = nc.dram_tensor("ma", [128, L, 3, SLICE], F16, kind="ExternalInput").ap()
    # stationary packs: [K=128 (2 taps x 64ch), pass, M=128]
    wqk = nc.dram_tensor("wqk", [128, 10, 128], F16, kind="ExternalInput").ap()
    wv = nc.dram_tensor("wv", [128, 14, 128], F16, kind="ExternalInput").ap()
    bq = nc.dram_tensor("bq", [128, 1], F32, kind="ExternalInput").ap()
    bk = nc.dram_tensor("bk", [128, 1], F32, kind="ExternalInput").ap()
    bv = nc.dram_tensor("bv", [128, 1], F32, kind="ExternalInput").ap()
    out = nc.dram_tensor("out", [128, LV * NPOS], F16, kind="ExternalOutput").ap()

    with tile.TileContext(nc) as tc:
        with tc.tile_pool(name="consts", bufs=1) as cpool, \
             tc.tile_pool(name="xin", bufs=4) as xin_pool, \
             tc.tile_pool(name="xmem", bufs=6) as xmem_pool, \
             tc.tile_pool(name="qkc", bufs=6) as qkc_pool, \
             tc.tile_pool(name="qkT", bufs=6) as qkT_pool, \
             tc.tile_pool(name="vall", bufs=1) as vall_pool, \
             tc.tile_pool(name="sm", bufs=2) as sm_pool, \
             tc.tile_pool(name="ost", bufs=3) as ost_pool, \
             tc.tile_pool(name="pconv", bufs=6, space="PSUM") as pconv, \
             tc.tile_pool(name="plogit", bufs=1, space="PSUM") as plogit:

            wqk_t = cpool.tile([128, 10, 128], F16)
            bq_t = cpool.tile([128, 1], F32)
            bk_t = cpool.tile([128, 1], F32)
            bv_t = cpool.tile([128, 1], F32)
            wv_t = cpool.tile([128, 14, 128], F16)
            # queue layout: nc.sync (SP) carries ONLY XBAR transposes (a mode
            # switch is a global DMA barrier; a barrier-blocked transpose must
            # not stall a queue with real work behind it). Weight loads and
            # output stores ride the Act HWDGE queue (copies only); per-slice
            # input loads ride the Pool SWDGE queue (separate completion-sem
            # ring, so transposes never queue behind a big input transfer).
            for t, d in [(wqk_t, wqk), (bq_t, bq), (bk_t, bk), (bv_t, bv),
                         (wv_t, wv)]:
                nc.scalar.dma_start(t[:], d)

            v_heads = [vall_pool.tile([128, DV * NPOS], F16, name=f"vh{h}")
                       for h in range(NH)]

            import contextlib
            rep_ctx = (tc.For_i(0, reps, 1) if reps > 1
                       else contextlib.nullcontext())
            with rep_ctx:
                logit_ps = [plogit.tile([128, 128], F32, tag="logit",
                                        name=f"logit{h}") for h in range(NH)]
                xa_w, ma_w = {}, {}

                def load_slice(l):
                    # Pool-engine SWDGE loads: keeps the HWDGE completion-sem
                    # ring free for the XBAR transposes (shared 8-lane ring
                    # otherwise stalls a transpose behind an input load)
                    xt = xin_pool.tile([128, 2, SLICE], F16, tag="xin",
                                       name="xin")
                    nc.gpsimd.dma_start(xt[:], xa[:, l])
                    mt = xmem_pool.tile([128, 3, SLICE], F16, tag="xmem",
                                        name="xmem")
                    nc.gpsimd.dma_start(mt[:], ma[:, l])
                    xa_w[l] = xt
                    ma_w[l] = mt

                def views(t, s, lo=0, hi=128):
                    """(lo:hi, section s) of a [128, n, SLICE] tile as p h w."""
                    return t[lo:hi, s].rearrange("p (h w) -> p h w", h=HP)

                def conv_q_tile(qp, xt, y0):
                    """5 matmuls: 3 T1 pairs, 1 T2 pair, 1 K=64 single (top)."""
                    t1 = views(xt, 0)
                    t2 = views(xt, 1)
                    t2t = views(xt, 1, 0, 64)
                    for dy in range(3):
                        nc.tensor.matmul(qp[:], wqk_t[:, dy],
                                         t1[:, y0 + dy:y0 + dy + 16, 0:32],
                                         start=(dy == 0), stop=False)
                    nc.tensor.matmul(qp[:], wqk_t[:, 3],
                                     t2[:, y0:y0 + 16, 0:32],
                                     start=False, stop=False)
                    nc.tensor.matmul(qp[:], wqk_t[0:64, 4],
                                     t2t[:, y0 + 2:y0 + 18, 0:32],
                                     start=False, stop=True)

                def conv_k_tile(kp, mt, y0):
                    """5 matmuls: 3 T1 pairs, 1 T2 pair, 1 K=64 single (bot:
                    T2 bottom holds P<<2,up1row, so rows y0+1 give tap (2,2))."""
                    t1 = views(mt, 0)
                    t2 = views(mt, 1)
                    t2b = views(mt, 1, 64, 128)
                    for dy in range(3):
                        nc.tensor.matmul(kp[:], wqk_t[:, 5 + dy],
                                         t1[:, y0 + dy:y0 + dy + 16, 0:32],
                                         start=(dy == 0), stop=False)
                    nc.tensor.matmul(kp[:], wqk_t[:, 8],
                                     t2[:, y0:y0 + 16, 0:32],
                                     start=False, stop=False)
                    nc.tensor.matmul(kp[:], wqk_t[64:128, 9],
                                     t2b[:, y0 + 1:y0 + 17, 0:32],
                                     start=False, stop=True)

                def conv_v_tile(vp, lv, y0):
                    """14 matmuls: 9 T1 pairs, 3 T2 pairs, 1 T3 cross-depth
                    pair ((0,2,2)+(1,2,2)), 1 K=64 single ((2,2,2))."""
                    for dl in range(3):
                        t1 = views(ma_w[lv + dl], 0)
                        for dy in range(3):
                            i = dl * 3 + dy
                            nc.tensor.matmul(vp[:], wv_t[:, i],
                                             t1[:, y0 + dy:y0 + dy + 16, 0:32],
                                             start=(i == 0), stop=False)
                    for dl in range(3):
                        t2 = views(ma_w[lv + dl], 1)
                        nc.tensor.matmul(vp[:], wv_t[:, 9 + dl],
                                         t2[:, y0:y0 + 16, 0:32],
                                         start=False, stop=False)
                    t3 = views(ma_w[lv], 2)
                    nc.tensor.matmul(vp[:], wv_t[:, 12],
                                     t3[:, y0 + 2:y0 + 18, 0:32],
                                     start=False, stop=False)
                    t2c = views(ma_w[lv + 2], 1, 0, 64)
                    nc.tensor.matmul(vp[:], wv_t[0:64, 13],
                                     t2c[:, y0 + 2:y0 + 18, 0:32],
                                     start=False, stop=True)

                def evict(dst, src, bias, use_act):
                    """PSUM -> SBUF fp16 with fused per-partition bias."""
                    if use_act:
                        nc.scalar.activation(dst, src, ACT.Identity,
                                             bias=bias)
                    else:
                        nc.vector.tensor_scalar_add(dst, src, bias)

                def conv_v_slice(lv, split_after_tile0=None):
                    """Both 16-row tiles of v output slice lv -> v_heads.
                    split_after_tile0: callback emitted between the tiles."""
                    vh, vd = lv // DV, lv % DV
                    for t in range(2):
                        vp = pconv.tile([128, 512], F32, tag="conv", name="vp")
                        conv_v_tile(vp, lv, t * 16)
                        evict(v_heads[vh][:, vd * NPOS + t * 512:
                                          vd * NPOS + (t + 1) * 512],
                              vp[:], bv_t[:], use_act=(t == 1))
                        if t == 0 and split_after_tile0 is not None:
                            split_after_tile0()

                def emit_logits(lslice, qkT):
                    hd = lslice // DQ
                    first = (lslice % DQ) == 0
                    last = (lslice % DQ) == DQ - 1
                    for j in range(8):
                        qs = slice(j * 128, (j + 1) * 128)
                        ks = slice(NPOS + j * 128, NPOS + (j + 1) * 128)
                        nc.tensor.matmul(
                            logit_ps[hd][:], qkT[:, qs], qkT[:, ks],
                            start=(first and j == 0),
                            stop=(last and j == 7),
                            skip_group_check=True)

                attnT = {}
                recips = {}

                def softmax_head(h):
                    """Unnormalized exp -> fp16 -> XBAR transpose; the 1/sum
                    scale is folded into the output evictions (shorter chain
                    before the attnT transpose can issue)."""
                    negmax = sm_pool.tile([128, 1], F32, tag="negmax",
                                          name="negmax")
                    nc.vector.tensor_reduce(negmax[:], logit_ps[h][:],
                                            op=mybir.AluOpType.max,
                                            axis=mybir.AxisListType.X,
                                            negate=True)
                    attn16 = sm_pool.tile([128, 128], F16, tag="attn16",
                                          name="attn16")
                    rowsum = sm_pool.tile([128, 1], F32, tag="rowsum",
                                          name="rowsum")
                    nc.scalar.activation(attn16[:], logit_ps[h][:],
                                         ACT.Exp, bias=negmax[:], scale=1.0,
                                         accum_out=rowsum[:])
                    aT = sm_pool.tile([128, 128], F16, tag="attnT",
                                      name="attnT")
                    nc.sync.dma_start(aT[:], attn16[:], transpose=True)
                    recip = sm_pool.tile([128, 1], F32, tag="recip",
                                         name="recip")
                    nc.vector.reciprocal(recip[:], rowsum[:])
                    attnT[h] = aT
                    recips[h] = recip

                ost_cur = {}

                def attn_chunks(h, cs, stops=(3, 7, 11, 15)):
                    """attn@v for chunks cs of head h; multi-chunk output
                    groups staged in SBUF then stored with a single DMA.
                    `stops` sets group boundaries (smaller final groups keep
                    the kernel tail short)."""
                    group0 = {}
                    for c in cs:
                        if h not in group0 or group0[h] is None:
                            group0[h] = c
                            ost_cur[h] = ost_pool.tile([128, 2048], F16,
                                                       tag="ost", name="ost")
                        po = pconv.tile([128, 512], F32, tag="conv", name="po")
                        nc.tensor.matmul(po[:], attnT[h][:],
                                         v_heads[h][:, c * 512:(c + 1) * 512],
                                         start=True, stop=True)
                        g = c - group0[h]
                        dsl = ost_cur[h][:, g * 512:(g + 1) * 512]
                        if c % 2 == 0:
                            nc.vector.tensor_scalar_mul(dsl, po[:],
                                                        recips[h][:])
                        else:
                            nc.scalar.activation(dsl, po[:], ACT.Identity,
                                                 scale=recips[h][:])
                        if c in stops:
                            off = h * DV * NPOS + group0[h] * 512
                            n = (g + 1) * 512
                            nc.scalar.dma_start(out[:, off:off + n],
                                                ost_cur[h][:, 0:n])
                            group0[h] = None

                load_slice(0)
                load_slice(1)
                pending = None
                for l in range(L):
                    xt, mt = xa_w[l], ma_w[l]

                    # q and k conv outputs share ONE [128, 2048] staging tile
                    # so a single XBAR transpose (one global-barrier mode
                    # switch) covers both
                    qkc = qkc_pool.tile([128, 2 * NPOS], F16, tag="qkc",
                                        name="qkc")
                    for t in range(2):
                        sl = slice(t * 512, (t + 1) * 512)
                        qp = pconv.tile([128, 512], F32, tag="conv", name="qp")
                        conv_q_tile(qp, xt, t * 16)
                        evict(qkc[:, sl], qp[:], bq_t[:], use_act=(t == 1))
                    for t in range(2):
                        sl = slice(NPOS + t * 512, NPOS + (t + 1) * 512)
                        kp = pconv.tile([128, 512], F32, tag="conv", name="kp")
                        conv_k_tile(kp, mt, t * 16)
                        evict(qkc[:, sl], kp[:], bk_t[:], use_act=(t == 1))

                    qkT = qkT_pool.tile([128, 2 * NPOS], F16, tag="qkT",
                                        name="qkT")
                    nc.sync.dma_start_transpose(
                        qkT[:].rearrange("p (j c) -> p j c", j=16), qkc[:])

                    # flush DEFERRED logits: depth 2 during pipeline fill
                    # (DMA engines are saturated with the first input loads,
                    # delaying the early transposes), depth 1 at steady state
                    pending.append((l, qkT))
                    depth = 2 if l < 6 else 1
                    while len(pending) > depth:
                        emit_logits(*pending.pop(0))

                    if l == 9:
                        # head-0 logits flushed above (slice 8): emit softmax
                        # + attnT BEFORE this iter's loads so the attnT XBAR
                        # transpose doesn't barrier-wait on their transfers
                        softmax_head(0)

                    # issue the NEXT loads only after the transposes: an XBAR
                    # mode switch is a global DMA barrier, so a transpose
                    # waits for every regular DMA issued before it — loads
                    # issued here gate iter l+1's transposes (plenty of slack)
                    # instead of this iter's.
                    if l + 2 < L:
                        load_slice(l + 2)

                    if l < L - 1:
                        if l >= 2:
                            conv_v_slice(l - 2)
                        if 10 <= l <= 13:
                            c0 = 4 * (l - 10)
                            attn_chunks(0, range(c0, c0 + 4))
                    else:
                        # l == 17: split lv=15 v-conv around the final logit
                        # flush + head-1 softmax, so the attnT XBAR latency
                        # hides behind v tile1's matmuls.
                        def _flush17():
                            emit_logits(*pending)
                            softmax_head(1)
                        conv_v_slice(15, split_after_tile0=_flush17)
                        pending = None
                        attn_chunks(1, range(16), stops=(3, 7, 11, 13, 15))
    nc.compile()
    return nc


def _shift_flat(flat, k):
    """flat [..., 1156] -> content shifted k positions earlier (zeros fill)."""
    out = np.zeros_like(flat)
    out[..., :SLICE - k] = flat[..., k:]
    return out


def prep_inputs(input, memory, wq, bq, wk, bk, wv, bv, **_legacy):
    """Host-side marshalling: fp16 shifted-copy image stages + weight packs."""
    input = np.asarray(input, dtype=np.float32)
    memory = np.asarray(memory, dtype=np.float32)
    wq = np.asarray(wq, dtype=np.float32) * DEPTH_SCALE
    bq = np.asarray(bq, dtype=np.float32) * DEPTH_SCALE
    wk = np.asarray(wk, dtype=np.float32)
    bk = np.asarray(bk, dtype=np.float32)
    wv = np.asarray(wv, dtype=np.float32)
    bv = np.asarray(bv, dtype=np.float32)

    def flat_padded(x):  # (B, CIN, L, SLICE) fp16
        p = np.zeros((B, CIN, L, HP, WP), np.float16)
        p[:, :, :, 1:H + 1, 1:W + 1] = x.astype(np.float16)
        return p.reshape(B, CIN, L, SLICE)

    def stage(flat, with_t3):
        # [B, 128, L, nsec, SLICE]
        nsec = 3 if with_t3 else 2
        st = np.zeros((B, 128, L, nsec, SLICE), np.float16)
        st[:, 0:64, :, 0] = flat
        st[:, 64:128, :, 0] = _shift_flat(flat, 1)
        t2top = _shift_flat(flat, 2)
        st[:, 0:64, :, 1] = t2top
        st[:, 64:128, :, 1] = _shift_flat(flat, HP + 2)
        if with_t3:
            st[:, 0:64, :, 2] = t2top
            st[:, 64:128, :L - 1, 2] = t2top[:, :, 1:]
        return st

    xa = stage(flat_padded(input), with_t3=False)
    ma = stage(flat_padded(memory), with_t3=True)

    def tap_qk(w, dy, dx):  # [64, 128] = (cin, cout)
        return w[:, :, 0, dy, dx].T

    wqk_p = np.zeros((128, 10, 128), np.float16)
    for dy in range(3):
        wqk_p[0:64, dy] = tap_qk(wq, dy, 0)
        wqk_p[64:128, dy] = tap_qk(wq, dy, 1)
        wqk_p[0:64, 5 + dy] = tap_qk(wk, dy, 0)
        wqk_p[64:128, 5 + dy] = tap_qk(wk, dy, 1)
    wqk_p[0:64, 3] = tap_qk(wq, 0, 2)
    wqk_p[64:128, 3] = tap_qk(wq, 1, 2)
    wqk_p[0:64, 4] = tap_qk(wq, 2, 2)
    wqk_p[0:64, 8] = tap_qk(wk, 0, 2)
    wqk_p[64:128, 8] = tap_qk(wk, 1, 2)
    wqk_p[64:128, 9] = tap_qk(wk, 2, 2)

    def tap_v(dl, dy, dx):
        return wv[:, :, dl, dy, dx].T

    wv_p = np.zeros((128, 14, 128), np.float16)
    for dl in range(3):
        for dy in range(3):
            wv_p[0:64, dl * 3 + dy] = tap_v(dl, dy, 0)
            wv_p[64:128, dl * 3 + dy] = tap_v(dl, dy, 1)
        wv_p[0:64, 9 + dl] = tap_v(dl, 0, 2)
        wv_p[64:128, 9 + dl] = tap_v(dl, 1, 2)
    wv_p[0:64, 12] = tap_v(0, 2, 2)
    wv_p[64:128, 12] = tap_v(1, 2, 2)
    wv_p[0:64, 13] = tap_v(2, 2, 2)

    shared = {
        "wqk": wqk_p, "wv": wv_p,
        "bq": bq.reshape(128, 1), "bk": bk.reshape(128, 1),
        "bv": bv.reshape(128, 1),
    }
    return [{"xa": np.ascontiguousarray(xa[b]),
             "ma": np.ascontiguousarray(ma[b]), **shared} for b in range(B)]


# legacy flags kept for test.py compatibility (ignored by build_module)
QK_F32R = True
SPLIT_LOGITS = False


def kernel(**inputs):
    if "nc" not in _CACHE:
        _CACHE["nc"] = build_module()
    nc = _CACHE["nc"]
    in_maps = prep_inputs(**inputs)
    res = bass_utils.run_bass_kernel_spmd(nc, in_maps, core_ids=list(range(B)))
    out = np.stack([res.results[b]["out"].reshape(COUT, LV, H, W)
                    for b in range(B)])
    return out.astype(np.float32)
